# revision 15
# baseline (speedup 1.0000x reference)
"""MLA (multi-latent attention) prefill kernel for Trainium2, 8 NeuronCores.

Tensor-parallel over heads for q / kv_b / w_o (each core owns 2 of 16
heads, host sums the o_proj partials), with the shared kv_a latent +
rope-key projection SEQUENCE-sharded: each core computes 256 of the 2048
positions (1/8 of the replicated work) and an on-device AllGather
broadcasts the latent + roped shared key + rmsnorm row-sums to all
cores, hidden behind the q projections.

Precision plan (harness gate is normalized max-err < 2e-2): the output
metric is dominated by the first ~100 queries, whose attention averages
only a few keys, so nothing feeding them may be fp8.  Chunk 0 (first 512
positions) therefore runs q in bf16, and the latent/kv_b path is bf16
everywhere (it feeds chunk-0 keys/values through the gather).  fp8
(e4m3, DoubleRow = 2x PE rate) is used where errors average out:
  - q projection for chunks 1-3 (weights x64 on host to dodge e4m3's
    2^-6 min-normal; the exp() scale argument dequants),
  - attnT-accumulate + softmax-sum for superblocks 1-3: exp writes
    probsT as fp8 into [128, 2, 512] PAIR tiles so both matmuls run
    DoubleRow over key-block pairs (v stored fp8 at 16x true scale via
    the rmsnorm rsqrt constants; the 1/16 rides the o-proj copies).
Attention superblock 0 keeps bf16 probs and a bf16 copy of chunk-0 v.
Scores are always bf16 (128-deep contraction cannot DoubleRow anyway).

Layouts are column-major [feature, seq] as in earlier revisions: scoresT
[key, query] so exp writes probsT straight to SBUF, softmax sums via
ones-matmul, rsqrt via quake seed + 2 Newton steps, causal masking via
gpsimd affine_select on diagonal key blocks only.
"""
import os
import sys
import types
import numpy as np
import ml_dtypes

import concourse.bass as bass
import concourse.mybir as mybir
import concourse.tile as tile
from concourse import bacc, bass_isa, bass_utils, masks

F32 = mybir.dt.float32
BF16 = mybir.dt.bfloat16
F8 = mybir.dt.float8e4
I32 = mybir.dt.int32

S, HID = 2048, 2048
H, NOPE, ROPE, VD, KLR = 16, 64, 64, 128, 512
QD = NOPE + ROPE          # 128
SCALE = QD ** -0.5
EPS = 1e-6
NCORES = 8
HPC = H // NCORES         # heads per core = 2

SC = 512                  # seq chunk for projections
NSC = S // SC             # 4
HC = HID // 128           # 16 hid chunks
SHW = S // NCORES         # 256: per-core kv_a seq shard width

WQ_S = 64.0               # q weights x64 for fp8 (dequant in exp scale)
KV_S = 16.0               # kT / v / attnT are 16x true (fp8 headroom)
EXP_SCALE = SCALE / (WQ_S * KV_S)
YT_PRE = 1.0 / (KV_S * KV_S)   # mt scale so rsqrt emits KV_S * rsqrt(mean+eps)
DR = mybir.MatmulPerfMode.DoubleRow


def build_nc():
    nc = bacc.Bacc("TRN2", target_bir_lowering=False, debug=False,
                   num_devices=NCORES)
    dr = {}
    dr["hT"] = nc.dram_tensor("hT", [HID, S], F8, kind="ExternalInput")
    dr["hTb0"] = nc.dram_tensor("hTb0", [HID, SC], BF16, kind="ExternalInput")
    dr["hTsh"] = nc.dram_tensor("hTsh", [HID, SHW], BF16, kind="ExternalInput")
    dr["wq"] = nc.dram_tensor("wq", [HID, 256], F8, kind="ExternalInput")
    dr["wqb"] = nc.dram_tensor("wqb", [HID, 256], BF16, kind="ExternalInput")
    dr["wkva"] = nc.dram_tensor("wkva", [HID, 576], BF16, kind="ExternalInput")
    dr["wkvb"] = nc.dram_tensor("wkvb", [KLR, 384], BF16, kind="ExternalInput")
    dr["wo"] = nc.dram_tensor("wo", [HPC * VD, HID], BF16, kind="ExternalInput")
    dr["cosq"] = nc.dram_tensor("cosq", [64, S], BF16, kind="ExternalInput")
    dr["msinq"] = nc.dram_tensor("msinq", [64, S], BF16, kind="ExternalInput")
    dr["cossh"] = nc.dram_tensor("cossh", [64, SHW], BF16, kind="ExternalInput")
    dr["msinsh"] = nc.dram_tensor("msinsh", [64, SHW], BF16,
                                  kind="ExternalInput")
    dr["o"] = nc.dram_tensor("o", [S, HID], BF16, kind="ExternalOutput")

    with tile.TileContext(nc) as tc:
        build_tile_kernel(nc, tc, {k: v.ap() for k, v in dr.items()})
    nc.compile()
    return nc


def build_tile_kernel(nc, tc, d):
    from contextlib import ExitStack
    with ExitStack() as ctx:
        _build_tile_kernel(nc, tc, d, ctx)


def _build_tile_kernel(nc, tc, d, ctx):
    AF = mybir.ActivationFunctionType
    ALU = mybir.AluOpType

    consts = ctx.enter_context(tc.tile_pool(name="consts", bufs=1))
    big = ctx.enter_context(tc.tile_pool(name="big", bufs=1))
    work = ctx.enter_context(tc.tile_pool(name="work", bufs=2))
    stat = ctx.enter_context(tc.tile_pool(name="stat", bufs=3))
    outp = ctx.enter_context(tc.tile_pool(name="outp", bufs=3))
    ps = ctx.enter_context(tc.tile_pool(name="ps", bufs=8, space="PSUM"))
    dram = ctx.enter_context(tc.tile_pool(name="dram", bufs=1, space="DRAM"))

    # collective bounce buffers as DRAM *tiles* so the tile framework
    # tracks write -> AllGather -> read ordering (raw dram_tensors are
    # invisible to its dependency analysis)
    lat_in = dram.tile([KLR + ROPE, SHW], BF16, tag="lat_in")
    lat_out = dram.tile([NCORES, KLR + ROPE, SHW], BF16, tag="lat_out")
    srow_in = dram.tile([1, SHW], F32, tag="srow_in")
    srow_out = dram.tile([1, S], F32, tag="srow_out")

    # ---- input DMAs ------------------------------------------------------
    # shard operands first: the latent shard + AllGather is the long pole
    hTsh_sb = consts.tile([128, HC, SHW], BF16)
    wkva_sb = consts.tile([128, HC, 576], BF16)
    cossh_sb = consts.tile([64, SHW], BF16)
    msinsh_sb = consts.tile([64, SHW], BF16)
    for kp in range(HC // 2):
        nc.sync.dma_start(
            out=hTsh_sb[:, 2 * kp:2 * kp + 2, :],
            in_=d["hTsh"][256 * kp:256 * (kp + 1), :].rearrange(
                "(k p) m -> p k m", p=128))
        nc.sync.dma_start(
            out=wkva_sb[:, 2 * kp:2 * kp + 2, :],
            in_=d["wkva"][256 * kp:256 * (kp + 1), :].rearrange(
                "(k p) m -> p k m", p=128))
    nc.sync.dma_start(out=cossh_sb[:], in_=d["cossh"])
    nc.sync.dma_start(out=msinsh_sb[:], in_=d["msinsh"])

    # chunk-0 bf16 q operands + the fp8 bulk on the other queue
    hTb0_sb = consts.tile([128, HC, SC], BF16)
    wqb_sb = consts.tile([128, HC, 256], BF16)
    wq_sb = consts.tile([128, HC, 256], F8)
    hT_sb = consts.tile([128, HC, S], F8)
    cos_sb = consts.tile([64, S], BF16)
    msin_sb = consts.tile([64, S], BF16)
    wkvb_sb = consts.tile([128, 4, 384], BF16)
    wo_sb = consts.tile([128, HPC, HID], BF16)
    for kp in range(HC // 2):
        nc.scalar.dma_start(
            out=wqb_sb[:, 2 * kp:2 * kp + 2, :],
            in_=d["wqb"][256 * kp:256 * (kp + 1), :].rearrange(
                "(k p) m -> p k m", p=128))
        nc.scalar.dma_start(
            out=hTb0_sb[:, 2 * kp:2 * kp + 2, :],
            in_=d["hTb0"][256 * kp:256 * (kp + 1), :].rearrange(
                "(k p) m -> p k m", p=128))
    nc.scalar.dma_start(out=cos_sb[:], in_=d["cosq"])
    nc.scalar.dma_start(out=msin_sb[:], in_=d["msinq"])
    for kp in range(HC // 2):
        nc.scalar.dma_start(
            out=wq_sb[:, 2 * kp:2 * kp + 2, :],
            in_=d["wq"][256 * kp:256 * (kp + 1), :].rearrange(
                "(k p) m -> p k m", p=128))
    # fp8 hT for chunks 1-3 (chunk 0 q runs bf16)
    for c in range(1, NSC):
        cs = slice(c * SC, (c + 1) * SC)
        for kp in range(HC // 2):
            nc.sync.dma_start(
                out=hT_sb[:, 2 * kp:2 * kp + 2, cs],
                in_=d["hT"][256 * kp:256 * (kp + 1), cs].rearrange(
                    "(k p) m -> p k m", p=128))
    nc.scalar.dma_start(out=wkvb_sb[:],
                        in_=d["wkvb"].rearrange("(k p) m -> p k m", p=128))
    nc.sync.dma_start(out=wo_sb[:], in_=d["wo"].rearrange("(h p) n -> p h n", p=128))

    ones_bf = consts.tile([128, 128], BF16)
    nc.vector.memset(ones_bf[:], 1.0)
    ones_f8p = consts.tile([128, 2, 128], F8)
    nc.vector.memset(ones_f8p[:], 1.0)
    one_f32 = consts.tile([1, 1], F32)
    nc.vector.memset(one_f32[:], 1.0)
    ident_f = consts.tile([128, 128], F32)
    masks.make_identity(nc, ident_f[:])
    ones_row = consts.tile([1, 128], BF16)
    nc.vector.memset(ones_row[:], 1.0)

    # ---- persistent activations -----------------------------------------
    qT = [big.tile([128, S], BF16, tag=f"qT{h}", name=f"qT{h}") for h in range(HPC)]
    kT = [big.tile([128, S], BF16, tag=f"kT{h}", name=f"kT{h}") for h in range(HPC)]
    v_sb = big.tile([128, S // 128, HPC * VD], F8, tag="v")
    # bf16 copy of v for chunk 0 (superblock 0's few-key queries)
    v0_bf = big.tile([128, 4, HPC * VD], BF16, tag="v0bf")
    srow_full = big.tile([1, S], F32, tag="srowf")

    # =====================================================================
    def shard_compute():
        """Per-core kv_a latent + roped shared key for this core's 256
        positions (bf16), plus the rmsnorm sum-of-squares row; shipped to
        DRAM and AllGathered."""
        # one full PSUM bank per m-block: a start=True reset is bank-wide,
        # so two interleaved accumulation groups must never share a bank
        pl = [ps.tile([128, 512], F32, tag="ps", name=f"pl{i}")
              for i in range(4)]
        pkpe = ps.tile([64, 512], F32, tag="ps", name="pkpesh")
        for k in range(HC):
            for m in range(4):
                nc.tensor.matmul(
                    pl[m][:, 0:SHW],
                    wkva_sb[:, k, m * 128:(m + 1) * 128],
                    hTsh_sb[:, k, :], start=(k == 0), stop=(k == HC - 1))
            nc.tensor.matmul(pkpe[:, 0:SHW], wkva_sb[:, k, 512:576],
                             hTsh_sb[:, k, :], start=(k == 0),
                             stop=(k == HC - 1))
        stg_sh = work.tile([128, 4, SHW], BF16, tag="stgsh", bufs=1)
        nc.vector.tensor_copy(stg_sh[:, 0, :], pl[0][:, 0:SHW])
        nc.vector.tensor_copy(stg_sh[:, 1, :], pl[1][:, 0:SHW])
        nc.scalar.copy(stg_sh[:, 2, :], pl[2][:, 0:SHW])
        nc.scalar.copy(stg_sh[:, 3, :], pl[3][:, 0:SHW])
        sq_sh = work.tile([128, 4, SHW], BF16, tag="sqsh", bufs=1)
        nc.scalar.activation(sq_sh[:], stg_sh[:], AF.Square)
        pssq = ps.tile([128, 512], F32, tag="ps", name="pssq")
        for m in range(4):
            nc.tensor.matmul(pssq[:, 0:SHW], ones_bf[:], sq_sh[:, m, :],
                             start=(m == 0), stop=(m == 3))
        srow_sh = stat.tile([1, SHW], F32, tag="srowsh", name="srowsh")
        nc.vector.tensor_copy(srow_sh[:], pssq[0:1, 0:SHW])
        # rope on the shard's shared key (x' in pkpe, true scale; the
        # cossh/msinsh tables carry KV_S so krope lands at KV_S x true)
        krope = work.tile([64, SHW], BF16, tag="krope", bufs=1)
        tks = work.tile([64, SHW], F32, tag="tks", bufs=2)
        tkc = work.tile([64, SHW], F32, tag="tks", bufs=2)
        nc.vector.tensor_tensor(tks[0:32, :], pkpe[32:64, 0:SHW],
                                msinsh_sb[32:64, :], ALU.mult)
        nc.vector.tensor_tensor(tks[32:64, :], pkpe[0:32, 0:SHW],
                                msinsh_sb[0:32, :], ALU.mult)
        nc.vector.tensor_tensor(tkc[:], pkpe[0:64, 0:SHW], cossh_sb[:],
                                ALU.mult)
        nc.vector.tensor_tensor(krope[:], tkc[:], tks[:], ALU.add)
        # ship shard to DRAM for the gather
        nc.sync.dma_start(
            out=lat_in[0:KLR, :].rearrange("(k p) m -> p k m", p=128),
            in_=stg_sh[:])
        nc.sync.dma_start(out=lat_in[KLR:KLR + ROPE, :], in_=krope[:])
        nc.sync.dma_start(out=srow_in[:], in_=srow_sh[:])
        grp = [list(range(NCORES))]
        nc.gpsimd.collective_compute(
            "AllGather", ALU.bypass, replica_groups=grp,
            ins=[lat_in.opt()], outs=[lat_out.opt()])
        nc.gpsimd.collective_compute(
            "AllGather", ALU.bypass, replica_groups=grp,
            ins=[srow_in.opt()], outs=[srow_out.opt()])
        nc.sync.dma_start(out=srow_full[:], in_=srow_out[:])

    def gather_in(c):
        """Pull chunk c's latent + roped key out of the gathered buffer."""
        stg = work.tile([128, 4, SC], BF16, tag="stg", bufs=2)
        for i, r in enumerate((2 * c, 2 * c + 1)):
            nc.sync.dma_start(
                out=stg[:, :, i * SHW:(i + 1) * SHW],
                in_=lat_out[r, 0:KLR, :].rearrange(
                    "(k p) m -> p k m", p=128))
            for h in range(HPC):
                nc.sync.dma_start(
                    out=kT[h][64:128, r * SHW:(r + 1) * SHW],
                    in_=lat_out[r, KLR:KLR + ROPE, :])
        return stg

    # =====================================================================
    def proj_w1(c):
        """q wave (bf16 for chunk 0, fp8 DoubleRow after) + rope epilogue."""
        cs = slice(c * SC, (c + 1) * SC)
        pq = [ps.tile([128, SC], F32, tag="ps", name=f"pq{i}") for i in range(HPC)]
        if c == 0:
            for k in range(HC):
                for h in range(HPC):
                    nc.tensor.matmul(pq[h][:],
                                     wqb_sb[:, k, h * 128:(h + 1) * 128],
                                     hTb0_sb[:, k, :], start=(k == 0),
                                     stop=(k == HC - 1))
        else:
            for k in range(HC // 2):
                kk = slice(2 * k, 2 * k + 2)
                for h in range(HPC):
                    nc.tensor.matmul(pq[h][:],
                                     wq_sb[:, kk, h * 128:(h + 1) * 128],
                                     hT_sb[:, kk, cs], start=(k == 0),
                                     stop=(k == HC // 2 - 1), perf_mode=DR)
        # rope: q' = x'*cos + y*sin, y = signed rotate-half of x'
        for h in range(HPC):
            nc.vector.tensor_copy(qT[h][0:64, cs], pq[h][0:64, :])
            t2 = work.tile([128, SC], F32, tag="t2", bufs=4)
            t3 = work.tile([128, SC], F32, tag="t2", bufs=4)
            nc.vector.tensor_tensor(t2[64:96, :], pq[h][96:128, :],
                                    msin_sb[32:64, cs], ALU.mult)
            nc.vector.tensor_tensor(t2[96:128, :], pq[h][64:96, :],
                                    msin_sb[0:32, cs], ALU.mult)
            nc.vector.tensor_tensor(t3[64:128, :], pq[h][64:128, :],
                                    cos_sb[0:64, cs], ALU.mult)
            nc.vector.tensor_tensor(qT[h][64:128, cs], t3[64:128, :],
                                    t2[64:128, :], ALU.add)

    # rmsnorm scale chain from the gathered row sums
    def stats_b1(c):
        """transpose row sums to [128,4] columns; quake rsqrt on them.
        YT_PRE folds the deliberate KV_S on kT/v into the constants."""
        pcol = ps.tile([128, 4], F32, tag="ps", name="pcolq")
        for qi in range(4):
            nc.tensor.transpose(pcol[:, qi:qi + 1],
                                srow_full[0:1, c * SC + qi * 128:
                                          c * SC + (qi + 1) * 128],
                                one_f32[0:1, 0:1])
        mt = stat.tile([128, 4], F32, tag="mt", name="mt")
        nc.vector.tensor_scalar(out=mt[:], in0=pcol[:], scalar1=YT_PRE / KLR,
                                scalar2=YT_PRE * EPS, op0=ALU.mult, op1=ALU.add)
        ti = stat.tile([128, 4], I32, tag="ti", name="ti")
        nc.vector.tensor_scalar(out=ti[:], in0=mt.bitcast(I32)[:],
                                scalar1=1, scalar2=None,
                                op0=ALU.logical_shift_right)
        yt = stat.tile([128, 4], F32, tag="yt", name="yt")
        nc.vector.tensor_scalar(out=yt.bitcast(I32)[:], in0=ti[:],
                                scalar1=-1, scalar2=0x5F3759DF,
                                op0=ALU.mult, op1=ALU.add)
        y2 = stat.tile([128, 4], F32, tag="y2", name="y2")
        for _ in range(2):
            nc.vector.tensor_tensor(y2[:], yt[:], yt[:], ALU.mult)
            nc.vector.scalar_tensor_tensor(out=y2[:], in0=y2[:], scalar=-0.5,
                                           in1=mt[:], op0=ALU.mult,
                                           op1=ALU.mult)
            nc.vector.scalar_tensor_tensor(out=yt[:], in0=y2[:], scalar=1.5,
                                           in1=yt[:], op0=ALU.add,
                                           op1=ALU.mult)
        return yt

    def stats_b2(yt):
        """columns back to a row; broadcast to all partitions."""
        prt = ps.tile([1, SC], F32, tag="ps", name="prtq")
        for qi in range(4):
            nc.tensor.transpose(prt[0:1, qi * 128:(qi + 1) * 128],
                                yt[:, qi:qi + 1], ident_f[:])
        rrbf = stat.tile([1, SC], BF16, tag="rrbfq", name="rrbfq")
        nc.vector.tensor_copy(rrbf[:], prt[0:1, :])
        sbcp = ps.tile([128, SC], F32, tag="ps", name="sbcp")
        nc.tensor.matmul(sbcp[:], ones_row[0:1, :], rrbf[0:1, :],
                         start=True, stop=True)
        sbcb = work.tile([128, SC], BF16, tag="sbcb", bufs=2)
        nc.scalar.copy(sbcb[:], sbcp[:])
        return sbcb

    def proj_mm2(c, stg, sbcb, yt):
        """kv_b (bf16) from the gathered latent; norm scale (with folded
        KV_S) applied at the kT and v writes."""
        cs = slice(c * SC, (c + 1) * SC)
        pnope = ps.tile([128, SC], F32, tag="ps", name="pnope")
        for kk in range(4):
            nc.tensor.matmul(pnope[:], wkvb_sb[:, kk, 0:128], stg[:, kk, :],
                             start=(kk == 0), stop=(kk == 3))
        for t in range(4):
            pv = ps.tile([128, HPC * VD], F32, tag="ps", name="pv")
            for kk in range(4):
                nc.tensor.matmul(pv[:], stg[:, kk, t * 128:(t + 1) * 128],
                                 wkvb_sb[:, kk, 128:384],
                                 start=(kk == 0), stop=(kk == 3))
            nc.vector.tensor_scalar_mul(v_sb[:, 4 * c + t, :], pv[:],
                                        yt[:, t:t + 1])
            if c == 0:
                nc.scalar.mul(v0_bf[:, t, :], pv[:], yt[:, t:t + 1])
        nc.vector.tensor_tensor(kT[0][0:64, cs], pnope[0:64, :],
                                sbcb[0:64, :], ALU.mult)
        nc.vector.tensor_tensor(kT[1][0:64, cs], pnope[64:128, :],
                                sbcb[64:128, :], ALU.mult)

    # =====================================================================
    def attn_core(B):
        """Transposed-scores attention for superblock B (512 queries), both
        heads.  B=0: bf16 probs/v (few-key queries).  B>=1: key blocks
        advance in PAIRS -- scoresT (PE, bf16) -> exp (ACT, fp8 probsT into
        a [128,2,512] pair tile) -> [causal zero via affine_select on
        diagonal key-blocks] -> attnT + ones-sum as fp8 DoubleRow matmuls.
        1/sumexp folds into the attnT PSUM->SBUF copy (or the o-proj for
        the tail superblock)."""
        npr = 2 * (B + 1)
        LAG = 1
        pa = [None, None]
        pone = [None, None]
        serow = [None, None]
        at = [None, None]
        for h in range(HPC):
            pa[h] = ps.tile([128, 512], F32, tag="ps", name=f"pa{h}")
            pone[h] = ps.tile([128, 512], F32, tag="ps", name=f"pone{h}")
            if B == 0:
                ptsb = {}
                for step in range(4 + LAG):
                    if step < 4:
                        kt = step
                        qoff = kt * 128
                        psc = ps.tile([128, 512], F32, tag="ps", name="psc")
                        nc.tensor.matmul(
                            psc[:, qoff:512],
                            kT[h][:, kt * 128:(kt + 1) * 128],
                            qT[h][:, qoff:512], start=True, stop=True)
                        pt = work.tile([128, 512], BF16, tag="pt0", bufs=3,
                                       name="pt0")
                        nc.scalar.activation(pt[:, qoff:512], psc[:, qoff:512],
                                             AF.Exp, scale=EXP_SCALE)
                        nc.gpsimd.affine_select(
                            out=pt[:, qoff:512], in_=pt[:, qoff:512],
                            compare_op=ALU.is_ge, fill=0.0,
                            base=0, channel_multiplier=-1,
                            pattern=[[1, 512 - qoff]])
                        ptsb[kt] = pt
                    if step >= LAG:
                        kt = step - LAG
                        pt = ptsb.pop(kt)
                        qo = kt * 128
                        nc.tensor.matmul(pa[h][:, qo:512],
                                         v0_bf[:, kt, h * VD:(h + 1) * VD],
                                         pt[:, qo:512], start=(kt == 0),
                                         stop=(kt == 3))
                        nc.tensor.matmul(pone[h][:, qo:512], ones_bf[:],
                                         pt[:, qo:512], start=(kt == 0),
                                         stop=(kt == 3))
                serow[h] = stat.tile([1, 512], F32, tag="serow", name="serow")
                nc.vector.tensor_copy(serow[h][:], pone[h][0:1, :])
                continue
            pts = {}
            for step in range(npr + LAG):
                if step < npr:
                    ptp = work.tile([128, 2, 512], F8, tag="pt", bufs=4,
                                    name="pt")
                    # pair-level query offset: both members compute/write
                    # [qo:512] so the DoubleRow moving slice is fully owned
                    # by this generation (member j=1's [qo:qo+128) garbage
                    # region is zero-filled by its affine_select)
                    qo = max(0, (2 * step - 4 * B) * 128)
                    for j in range(2):
                        kt = 2 * step + j
                        psc = ps.tile([128, 512], F32, tag="ps", name="psc")
                        nc.tensor.matmul(
                            psc[:, qo:512],
                            kT[h][:, kt * 128:(kt + 1) * 128],
                            qT[h][:, B * 512 + qo:(B + 1) * 512],
                            start=True, stop=True)
                        nc.scalar.activation(ptp[:, j, qo:512],
                                             psc[:, qo:512],
                                             AF.Exp, scale=EXP_SCALE)
                        if kt >= 4 * B:
                            # zero probs where query < key
                            nc.gpsimd.affine_select(
                                out=ptp[:, j, qo:512], in_=ptp[:, j, qo:512],
                                compare_op=ALU.is_ge,
                                fill=0.0, base=B * 512 - kt * 128 + qo,
                                channel_multiplier=-1,
                                pattern=[[1, 512 - qo]])
                    pts[step] = ptp
                if step >= LAG:
                    pr = step - LAG
                    ptp = pts.pop(pr)
                    qo = max(0, (2 * pr - 4 * B) * 128)
                    nc.tensor.matmul(pa[h][:, qo:512],
                                     v_sb[:, 2 * pr:2 * pr + 2,
                                          h * VD:(h + 1) * VD],
                                     ptp[:, :, qo:512], start=(pr == 0),
                                     stop=(pr == npr - 1), perf_mode=DR)
                    nc.tensor.matmul(pone[h][:, qo:512], ones_f8p[:],
                                     ptp[:, :, qo:512], start=(pr == 0),
                                     stop=(pr == npr - 1), perf_mode=DR)
            serow[h] = stat.tile([1, 512], F32, tag="serow", name="serow")
            if B == NSC - 1:
                # tail folds 1/sum into oproj; pa is KV_S x true there, so
                # bake the 1/KV_S into the reciprocal's input
                nc.vector.tensor_scalar_mul(serow[h][:], pone[h][0:1, :], KV_S)
            else:
                nc.vector.tensor_copy(serow[h][:], pone[h][0:1, :])
        # ---- normalization epilogue for both heads ----
        pcol = [None, None]
        for h in range(HPC):
            pcol[h] = ps.tile([128, 4], F32, tag="ps", name=f"pcol{h}")
            for qi in range(4):
                nc.tensor.transpose(pcol[h][:, qi:qi + 1],
                                    serow[h][0:1, qi * 128:(qi + 1) * 128],
                                    one_f32[0:1, 0:1])
        rb4 = [None, None]
        for h in range(HPC):
            rb4[h] = stat.tile([128, 4], F32, tag="r4", name="r4")
            nc.vector.reciprocal(rb4[h][:], pcol[h][:])
        if B == NSC - 1:
            # tail: nothing follows to hide the broadcast chain -- ship the
            # attnT unscaled and fold 1/sumexp into the o_proj epilogue as
            # per-partition column scales (rb4 is already [q-tile, 1] form)
            for h in range(HPC):
                a = work.tile([128, 512], BF16, tag=f"at{h}", name=f"at{h}")
                nc.vector.tensor_copy(a[:], pa[h][:])
                at[h] = a
            return at, rb4
        prt = [None, None]
        for h in range(HPC):
            prt[h] = ps.tile([1, 512], F32, tag="ps", name=f"prt{h}")
            for qi in range(4):
                nc.tensor.transpose(prt[h][0:1, qi * 128:(qi + 1) * 128],
                                    rb4[h][:, qi:qi + 1], ident_f[:])
        for h in range(HPC):
            rrbf = stat.tile([1, 512], BF16, tag="rrbf", name="rrbf")
            nc.vector.tensor_copy(rrbf[:], prt[h][0:1, :])
            rbp = ps.tile([128, 512], F32, tag="ps", name="rbp")
            nc.tensor.matmul(rbp[:], ones_row[0:1, :], rrbf[0:1, :],
                             start=True, stop=True)
            rbc = work.tile([128, 512], BF16, tag="rbc")
            nc.scalar.copy(rbc[:], rbp[:])
            a = work.tile([128, 512], BF16, tag=f"at{h}", name=f"at{h}")
            # at = pa * (1/sum): pa is KV_S x true, at stays KV_S x; the
            # 1/KV_S lands in the oproj output copies
            nc.vector.tensor_tensor(a[:], pa[h][:], rbc[:], ALU.mult)
            at[h] = a
        return at, None

    def attn_oproj(B, at, rb4=None):
        for t in range(4):
            ot = outp.tile([128, 4, 512], BF16, tag="ot")
            for n in range(4):
                if rb4 is None:
                    po = ps.tile([128, 512], F32, tag="ps", name="po")
                    for h in range(HPC):
                        nc.tensor.matmul(po[:],
                                         at[h][:, t * 128:(t + 1) * 128],
                                         wo_sb[:, h, n * 512:(n + 1) * 512],
                                         start=(h == 0), stop=(h == HPC - 1))
                    if n % 2 == 0:
                        nc.vector.tensor_scalar_mul(ot[:, n, :], po[:],
                                                    1.0 / KV_S)
                    else:
                        nc.scalar.mul(ot[:, n, :], po[:], 1.0 / KV_S)
                else:
                    po0 = ps.tile([128, 512], F32, tag="ps", name="po0")
                    po1 = ps.tile([128, 512], F32, tag="ps", name="po1")
                    nc.tensor.matmul(po0[:], at[0][:, t * 128:(t + 1) * 128],
                                     wo_sb[:, 0, n * 512:(n + 1) * 512],
                                     start=True, stop=True)
                    nc.tensor.matmul(po1[:], at[1][:, t * 128:(t + 1) * 128],
                                     wo_sb[:, 1, n * 512:(n + 1) * 512],
                                     start=True, stop=True)
                    t4 = work.tile([128, 512], BF16, tag="t4", bufs=3)
                    nc.scalar.mul(t4[:], po0[:], rb4[0][:, t:t + 1])
                    nc.vector.scalar_tensor_tensor(
                        out=ot[:, n, :], in0=po1[:],
                        scalar=rb4[1][:, t:t + 1], in1=t4[:],
                        op0=ALU.mult, op1=ALU.add)
            nc.sync.dma_start(
                out=d["o"][(4 * B + t) * 128:(4 * B + t + 1) * 128, :],
                in_=ot[:])

    # =====================================================================
    # schedule: shard+gather kicks off first and hides behind the q waves;
    # each chunk's kv_b + rmsnorm chain then slots between attention
    # superblocks.
    shard_compute()
    proj_w1(0)
    proj_w1(1)
    proj_w1(2)
    proj_w1(3)

    stg = gather_in(0)
    yt = stats_b1(0)
    sbcb = stats_b2(yt)
    proj_mm2(0, stg, sbcb, yt)

    for c in range(1, NSC):
        at, rb4t = attn_core(c - 1)
        stg = gather_in(c)
        yt = stats_b1(c)
        sbcb = stats_b2(yt)
        proj_mm2(c, stg, sbcb, yt)
        attn_oproj(c - 1, at, rb4t)
    at, rb4t = attn_core(NSC - 1)
    attn_oproj(NSC - 1, at, rb4t)


# =========================================================================
# host side
# =========================================================================
_perm1 = np.concatenate([np.arange(0, ROPE, 2), np.arange(1, ROPE, 2)])


def _host_prep(inputs):
    hidden = np.ascontiguousarray(np.asarray(inputs["hidden_states"],
                                             dtype=np.float32)[0])
    cos = np.asarray(inputs["cos"], dtype=np.float32)[0]
    sin = np.asarray(inputs["sin"], dtype=np.float32)[0]
    w_q = np.asarray(inputs["w_q"], dtype=np.float32)
    w_kv_a = np.asarray(inputs["w_kv_a"], dtype=np.float32)
    ln_w = np.asarray(inputs["kv_a_ln_w"], dtype=np.float32)
    w_kv_b = np.asarray(inputs["w_kv_b"], dtype=np.float32)
    w_o = np.asarray(inputs["w_o"], dtype=np.float32)

    hT = np.ascontiguousarray(hidden.T)
    cosT = cos.T
    sinT = sin.T
    # q-side rope tables (full scale); k-side shard tables carry KV_S
    cosq = np.ascontiguousarray(cosT)
    msinq = np.ascontiguousarray(np.concatenate(
        [sinT[32:64], -sinT[0:32]], axis=0))
    cosk = cosT * KV_S
    msink = np.concatenate([sinT[32:64], -sinT[0:32]], axis=0) * KV_S

    kpe_cols = w_kv_a[:, KLR:]
    kpe_x = kpe_cols[:, _perm1]
    wkva_mod = np.ascontiguousarray(np.concatenate(
        [w_kv_a[:, :KLR], kpe_x], axis=1))                   # [HID, 576]
    wkvb_all = w_kv_b * ln_w[:, None]

    bf = ml_dtypes.bfloat16
    f8 = ml_dtypes.float8_e4m3
    in_maps = []
    for cid in range(NCORES):
        heads = [HPC * cid + i for i in range(HPC)]
        blocks = []
        for h in heads:
            wq_h = w_q[:, h * QD:(h + 1) * QD]
            blocks.append(np.concatenate(
                [wq_h[:, :NOPE], wq_h[:, NOPE:][:, _perm1]], axis=1))
        wq_mod = np.ascontiguousarray(np.concatenate(blocks, axis=1) * WQ_S)

        nope_b = [wkvb_all[:, h * (NOPE + VD):h * (NOPE + VD) + NOPE]
                  for h in heads]
        v_b = [wkvb_all[:, h * (NOPE + VD) + NOPE:(h + 1) * (NOPE + VD)]
               for h in heads]
        wkvb_mod = np.ascontiguousarray(np.concatenate(nope_b + v_b, axis=1))

        wo_mod = np.ascontiguousarray(w_o[heads[0] * VD:(heads[-1] + 1) * VD, :])

        sh = slice(cid * SHW, (cid + 1) * SHW)
        in_maps.append({
            "hT": hT.astype(f8),
            "hTb0": np.ascontiguousarray(hT[:, 0:SC]).astype(bf),
            "hTsh": np.ascontiguousarray(hT[:, sh]).astype(bf),
            "wq": wq_mod.astype(f8), "wqb": wq_mod.astype(bf),
            "wkva": wkva_mod.astype(bf), "wkvb": wkvb_mod.astype(bf),
            "wo": wo_mod.astype(bf),
            "cosq": cosq.astype(bf), "msinq": msinq.astype(bf),
            "cossh": np.ascontiguousarray(cosk[:, sh]).astype(bf),
            "msinsh": np.ascontiguousarray(msink[:, sh]).astype(bf)})
    return in_maps


def _install_ntff_hook():
    """Make trace=True work under axon (antenv.axon_hooks is absent in this
    image; back it with trn_agent_boot's ctypes hook)."""
    try:
        import antenv
        if "antenv.axon_hooks" in sys.modules:
            return
        from trn_agent_boot.trn_boot import _ntff_profile_via_ctypes
        hook = _ntff_profile_via_ctypes("/opt/axon/libaxon_pjrt.so")
        mod = types.ModuleType("antenv.axon_hooks")
        mod.get_axon_ntff_profile_hook = lambda: hook
        mod.set_axon_ntff_profile_hook = lambda h: None
        sys.modules["antenv.axon_hooks"] = mod
        antenv.axon_hooks = mod
    except Exception:
        pass


_nc_cache = None
last_results = None


def kernel(**inputs):
    global _nc_cache, last_results
    _install_ntff_hook()
    if _nc_cache is None:
        _nc_cache = build_nc()
    in_maps = _host_prep(inputs)
    trace = bool(os.environ.get("BASS_TRACE"))
    res = bass_utils.run_bass_kernel_spmd(
        _nc_cache, in_maps, core_ids=list(range(NCORES)), trace=trace)
    last_results = res
    total = res.results[0]["o"].astype(np.float32)
    for c in range(1, NCORES):
        total = total + res.results[c]["o"]
    return total.reshape(1, S, HID)
